# revision 20
# baseline (speedup 1.0000x reference)
"""Trainium2 Bass kernel for nn_Lookback: causal running-mean over T.

out[b, t, c] = (1/(t+1)) * sum_{s<=t} x[b, s, c],  x: [8, 4096, 1024] fp32.

Sharding: data-parallel over batch B — core b handles x[b] ([4096, 1024]).

Per-core algorithm (T tiled into 32 blocks of P=128 rows, processed as 16
pairs; partition-REVERSED outputs: ps[p] holds global row 128k + 127 - p,
so each tile's running total lands on partition 0):
  For pair m (tiles a=2m, b=2m+1):
    ps_a = flipT @ x_a            (+ ones1 @ carry[m-1] for m>0)
    ps_b = flipT @ x_b + ones128 @ x_a   (+ ones1 @ carry[m-1])
    carry[m] = ps_b[0]            (partition 0 = running total through
                                   tile b; one DVE extract per PAIR — the
                                   serial carry chain at half frequency)
    out_{a,b} = ps_{a,b} * 1/(t+1)  (ACT eviction, per-partition scale, bf16)
  All matmuls run in bf16 (1 cyc/row; fp32r measures ~2-3 cyc/row on this
  part, and the PE clock is duty-cycle throttled, so bf16 halves PE time).
  x is cast f32->bf16 on-chip (DVE/ACT) through a staging ring; input
  precision bf16 is ~0.2% — tolerance is 2e-2.
  The host un-reverses each 128-row block during the gather (numpy view).
  Output is stored as bf16, halving store traffic: 16 MiB loads + 8 MiB
  stores ~= 60-70us HBM floor per core at ~400 GB/s.

Engine split: SP issues loads (queue 1), GPSIMD issues stores (queue 0 —
separate queue so stores never block loads), DVE casts + extracts carry,
ACT casts + evicts, PE does matmuls only.
"""

import sys

import numpy as np

sys.path.insert(0, "/opt/trn_rl_repo")

import concourse.bass as bass
import concourse.mybir as mybir
import concourse.tile as tile
from concourse import bacc
from concourse.bass_utils import run_bass_kernel_spmd

B, T, C = 8, 4096, 1024
P = 128
NT = T // P          # 32 row tiles per core
NP = NT // 2         # 16 pairs
CH = 512             # PSUM bank chunk (fp32)
NCH = C // CH
F32 = mybir.dt.float32
BF16 = mybir.dt.bfloat16

_cache = {}


def _consts():
    """Host-precomputed weight matrices (shared by all cores)."""
    # flipT[q, p] = [q <= 127 - p]: out partition p = global row 128k+127-p
    flip_t = np.triu(np.ones((P, P), np.float32))[:, ::-1].copy()
    ones1 = np.ones((1, P), np.float32)
    ones128 = np.ones((P, P), np.float32)
    # recip[p, k] = 1 / (128*k + 127 - p + 1)
    pidx = np.arange(P, dtype=np.float64)[:, None]      # [P, 1]
    kidx = np.arange(NT, dtype=np.float64)[None, :]     # [1, NT]
    recip = (1.0 / (128.0 * kidx + 128.0 - pidx)).astype(np.float32)
    import ml_dtypes
    bf = lambda a: a.astype(ml_dtypes.bfloat16)
    return bf(flip_t), bf(ones1), bf(ones128), recip


def _build():
    nc = bacc.Bacc("TRN2", target_bir_lowering=False, debug=False, num_devices=B)
    x_d = nc.dram_tensor("x", [T, C], F32, kind="ExternalInput").ap()
    flip_d = nc.dram_tensor("flip_t", [P, P], BF16, kind="ExternalInput").ap()
    ones1_d = nc.dram_tensor("ones1", [1, P], BF16, kind="ExternalInput").ap()
    ones128_d = nc.dram_tensor("ones128", [P, P], BF16, kind="ExternalInput").ap()
    r_d = nc.dram_tensor("recip", [P, NT], F32, kind="ExternalInput").ap()
    out_d = nc.dram_tensor("out", [T, C], BF16, kind="ExternalOutput").ap()

    x_pn = x_d.rearrange("(n p) c -> p n c", p=P)                # [P, NT, C]
    out_g = out_d.rearrange("(m n p) c -> m p n c", p=P, n=2)    # [16, P, 2, C]

    with tile.TileContext(nc) as tc:
        with (
            tc.tile_pool(name="const", bufs=1) as cp,
            tc.tile_pool(name="stg", bufs=8) as sp,
            tc.tile_pool(name="xbf", bufs=1) as xp,
            tc.tile_pool(name="carry", bufs=1) as kp,
            tc.tile_pool(name="ev", bufs=4) as ep,
            tc.tile_pool(name="ps", bufs=4, space=bass.MemorySpace.PSUM) as psp,
        ):
            flip_s = cp.tile([P, P], BF16)
            ones1_s = cp.tile([1, P], BF16)
            ones128_s = cp.tile([P, P], BF16)
            r_s = cp.tile([P, NT], F32)
            nc.sync.dma_start(flip_s[:], flip_d)
            nc.sync.dma_start(ones1_s[:], ones1_d)
            nc.sync.dma_start(ones128_s[:], ones128_d)
            nc.sync.dma_start(r_s[:], r_d)

            xr = xp.tile([P, NT, C], BF16)            # bf16 resident input
            carry = kp.tile([1, 2, C], BF16)          # running-total row, 2 slots

            # all loads up-front on the sync queue; stage ring (bufs=4)
            # throttles them to stay <= 4 pairs ahead of the casts
            stage = []
            for g in range(NP):
                st = sp.tile([P, 2, C], F32, tag="stg")
                nc.sync.dma_start(st[:], x_pn[:, 2 * g:2 * g + 2, :])
                stage.append(st)

            def cast_pair(g):
                nc.vector.tensor_copy(xr[:, 2 * g, :], stage[g][:, 0, :])
                nc.vector.tensor_copy(xr[:, 2 * g + 1, :], stage[g][:, 1, :])

            # PE warm-up while the first loads+casts land
            wu = psp.tile([P, C], F32, tag="ps")
            for _ in range(6):
                nc.tensor.matmul(wu[:, 0:P], flip_s[:], flip_s[:],
                                 start=True, stop=True)

            cast_pair(0)
            cast_pair(1)

            for m in range(NP):
                a, b = 2 * m, 2 * m + 1
                if m + 2 < NP:
                    cast_pair(m + 2)
                xa = xr[:, a, :]
                xb = xr[:, b, :]
                ps_a = psp.tile([P, C], F32, tag="ps")
                ps_b = psp.tile([P, C], F32, tag="ps")
                # bf16 moving operands allow N=1024 single matmuls
                nc.tensor.matmul(ps_a[:], flip_s[:], xa,
                                 start=True, stop=(m == 0))
                nc.tensor.matmul(ps_b[:], flip_s[:], xb,
                                 start=True, stop=False)
                nc.tensor.matmul(ps_b[:], ones128_s[:], xa,
                                 start=False, stop=(m == 0))
                if m > 0:
                    # carry-dependent matmuls last: ps_b first (the extract
                    # chain hangs off it), ps_a off the critical path
                    nc.tensor.matmul(ps_b[:], ones1_s[:],
                                     carry[:, (m - 1) % 2, :],
                                     start=False, stop=True)
                    nc.tensor.matmul(ps_a[:], ones1_s[:],
                                     carry[:, (m - 1) % 2, :],
                                     start=False, stop=True)
                # extract running total (partition 0) for the next pair
                if m < NP - 1:
                    nc.vector.tensor_copy(carry[:, m % 2, :], ps_b[0:1, :])
                # scaled evictions to bf16
                o = ep.tile([P, 2, C], BF16, tag="o")
                if m == NP - 1:
                    # final pair: evict in parallel on ACT+DVE, store each
                    # half as soon as it is ready to shorten the tail
                    nc.scalar.activation(
                        o[:, 1, :], ps_b[:], mybir.ActivationFunctionType.Copy,
                        scale=r_s[:, b:b + 1],
                    )
                    nc.vector.tensor_scalar_mul(o[:, 0, :], ps_a[:],
                                                r_s[:, a:a + 1])
                    nc.gpsimd.dma_start(out_g[m][:, 1, :], o[:, 1, :])
                    nc.gpsimd.dma_start(out_g[m][:, 0, :], o[:, 0, :])
                else:
                    nc.scalar.activation(
                        o[:, 0, :], ps_a[:], mybir.ActivationFunctionType.Copy,
                        scale=r_s[:, a:a + 1],
                    )
                    nc.scalar.activation(
                        o[:, 1, :], ps_b[:], mybir.ActivationFunctionType.Copy,
                        scale=r_s[:, b:b + 1],
                    )
                    nc.gpsimd.dma_start(out_g[m], o[:])

    nc.compile()
    return nc


def _run(x, trace=False):
    x = np.ascontiguousarray(x, dtype=np.float32)
    assert x.shape == (B, T, C)
    if "nc" not in _cache:
        _cache["nc"] = _build()
        _cache["consts"] = _consts()
    nc = _cache["nc"]
    flip_t, ones1, ones128, recip = _cache["consts"]
    in_maps = [
        {"x": x[b], "flip_t": flip_t, "ones1": ones1, "ones128": ones128,
         "recip": recip}
        for b in range(B)
    ]
    res = run_bass_kernel_spmd(nc, in_maps, core_ids=list(range(B)), trace=trace)
    # un-reverse each 128-row block (device wrote them partition-flipped)
    out = np.stack([
        np.asarray(res.results[b]["out"])
        .reshape(NT, P, C)[:, ::-1, :]
        .reshape(T, C)
        .astype(np.float32)
        for b in range(B)
    ])
    return out, res


def kernel(x):
    out, _ = _run(x, trace=False)
    return out
